# revision 35
# baseline (speedup 1.0000x reference)
"""Trainium2 Bass kernel for nn_Decoder_22273700397266 (sparse_attention).

Reference math (B=64, L=512, H=X=1024, D=2048):
    a   = concat(h_state, x, -1)                         # [B, L, D]
    s   = h_state.sum(axis=1)                            # [B, H]
    et  = tanh(a @ W_a.T + b_a + (s @ W_s.T + b_s)[:,None,:])
    et  = where(mask==0, -1e9, et)
    attn = softmax(et, axis=-1)                          # over D
    out = (a[b, trigger[b]] * attn).sum(axis=1)          # [B, D]

Sharding: data-parallel over batch across 8 NeuronCores (8 batches/core);
W_a / W_s replicated.

Per-core kernel design:
  * main matmul z = a @ W_a.T in bf16: rows on PSUM partitions, features on
    the free dim (the softmax axis), K-chunked over 16x128; W_a^T resident in
    SBUF (4 sub-tiles so matmuls start as soon as the first DMA quarter lands).
  * per-batch bias vector bias_b = b_a + b_s + W_s @ s_b:
      - s_b = sum_l h[b,l,:] via DVE reduces over an fp16 h stream (SWDGE
        queue so it never blocks the main DMA stream),
      - fp16 matvec on PE, bf16 hi+lo split (keeps |bias|~13 to ~2e-4),
      - folded into the z PSUM accumulation with one rank-2 ones-matmul.
  * the first batch's 4 row blocks cannot wait for the bias (it needs the
    whole h stream): their z groups close WITHOUT bias, z is stashed to SBUF
    (fp16), and once the bias exists it is applied as a broadcast [128, D]
    tile (ones-matmul into PSUM -> f32 SBUF) with a DVE add. This keeps the
    PE dense from ~8us onward instead of idling ~90us on the bias path.
  * tanh in [-1,1] -> softmax needs no max subtraction; masking folds into
    the exp as p = exp(mask * t): masked rows give exp(0)=1 everywhere =
    exactly the uniform softmax the reference produces for -1e9 rows.
  * denominator S = sum_e p comes free from the exp op's accum_out.
  * sum_l attn is a second matmul: acc[8, e] += w_wide.T @ p with
    w_wide[:, b] = 1/S (zero elsewhere), accumulated over all 32 row blocks.
  * out = acc * trig_a (host-gathered trigger rows), one [8, 2048] DMA out.
"""

import numpy as np
import ml_dtypes

import concourse.bass as bass
import concourse.bacc as bacc
import concourse.mybir as mybir
import concourse.tile as tile
from concourse.bass_utils import run_bass_kernel_spmd

F32 = mybir.dt.float32
F16 = mybir.dt.float16
BF16 = mybir.dt.bfloat16
AF = mybir.ActivationFunctionType
ALU = mybir.AluOpType
AX = mybir.AxisListType

B, L, H, X = 64, 512, 1024, 1024
D = H + X  # 2048
N_CORES = 8
B_SHARD = B // N_CORES  # 8

# Diagnostics stashed by kernel() for test harnesses.
LAST_RESULT = None


def build_program(b_shard: int = B_SHARD):
    """Build the per-core Bass program. Same program runs SPMD on all cores."""
    rows = b_shard * L              # 4096
    n_rb = rows // 128              # 32 row blocks of 128 rows
    rb_per_batch = L // 128         # 4
    n_kc = D // 128                 # 16 contraction chunks for the main matmul
    n_dc = H // 128                 # 8 contraction chunks for the s matvec
    n_ec = D // 512                 # 4 feature chunks of 512

    n_stash = rb_per_batch          # batch 0's row blocks take the late-bias path

    nc = bacc.Bacc("TRN2", target_bir_lowering=False, debug=False)

    a_t = nc.dram_tensor("a_t", [n_rb, 128, n_kc, 128], BF16, kind="ExternalInput").ap()
    w_t = nc.dram_tensor("w_t", [n_kc, 128, D], BF16, kind="ExternalInput").ap()
    h_t16 = nc.dram_tensor("h_t16", [n_dc, 128, rows], F16, kind="ExternalInput").ap()
    ws_t = nc.dram_tensor("ws_t", [n_dc, 128, D], F16, kind="ExternalInput").ap()
    bconst = nc.dram_tensor("bconst", [1, D], F16, kind="ExternalInput").ap()
    maskf = nc.dram_tensor("maskf", [128, n_rb], F32, kind="ExternalInput").ap()
    trig = nc.dram_tensor("trig", [b_shard, D], F32, kind="ExternalInput").ap()
    out = nc.dram_tensor("out", [b_shard, D], F32, kind="ExternalOutput").ap()

    with tile.TileContext(nc) as tc:
        with (
            tc.tile_pool(name="consts", bufs=1) as consts,
            tc.tile_pool(name="hpool", bufs=2) as hpool,
            tc.tile_pool(name="wspool", bufs=2) as wspool,
            tc.tile_pool(name="b2pool", bufs=3) as b2pool,
            tc.tile_pool(name="small32", bufs=2) as small32,
            tc.tile_pool(name="apool", bufs=3) as apool,
            tc.tile_pool(name="ppool", bufs=3) as ppool,
            tc.tile_pool(name="pstash", bufs=n_stash) as pstash,
            tc.tile_pool(name="zstash", bufs=n_stash) as zstash,
            tc.tile_pool(name="tpool", bufs=6) as tpool,
            tc.tile_pool(name="spool", bufs=6) as spool,
            tc.tile_pool(name="zpool", bufs=2, space="PSUM") as zpool,
            tc.tile_pool(name="accpool", bufs=1, space="PSUM") as accpool,
        ):
            # ---- resident constants -------------------------------------
            mask_sb = consts.tile([128, n_rb], F32, tag="mask_sb")
            nc.sync.dma_start(out=mask_sb, in_=maskf)

            # W_a^T in 8 sub-tiles, all on the sync queue, interleaved with
            # the first a tiles: the PE's first matmuls start as soon as the
            # first W chunk lands (~13us incl the ~8us DMA startup) and are
            # then paced by the W stream until ~30us.
            kq = n_kc // 8
            w_sbs = []
            w_aps = [
                w_t.rearrange("kc p e -> p kc e")[:, q * kq:(q + 1) * kq, :]
                for q in range(8)
            ]
            a_sb_pre = {}

            def pre_a(rb):
                a_sb_pre[rb] = apool.tile(
                    [128, n_kc, 128], BF16, tag="a_sb", name=f"a_sb{rb}"
                )
                nc.sync.dma_start(out=a_sb_pre[rb], in_=a_t[rb])

            pre_a(0)
            for q in range(8):
                w_q = consts.tile([128, kq, D], BF16, tag=f"w_sb{q}", name=f"w_sb{q}")
                nc.sync.dma_start(out=w_q, in_=w_aps[q])
                w_sbs.append(w_q)
                if q == 1:
                    pre_a(1)
                if q == 3:
                    pre_a(2)

            def w_slice(kc, sl):
                return w_sbs[kc // kq][:, kc % kq, sl]

            # tiny DMA depending on the last W chunk: sequences the gpsimd
            # queue so the h stream only starts once the W/a prefetch has had
            # the full HBM bandwidth (h isn't consumed until ~55us anyway)
            hgate = consts.tile([1, 2], BF16, tag="hgate")
            nc.gpsimd.dma_start(out=hgate, in_=w_sbs[7][0:1, 0, 0:2])

            ones2 = consts.tile([2, 128], BF16, tag="ones2")
            nc.vector.memset(ones2, 1.0)
            ones1 = consts.tile([1, b_shard], F16, tag="ones1")
            nc.vector.memset(ones1, 1.0)

            bconst_sb = consts.tile([1, D], F16, tag="bconst_sb")
            nc.gpsimd.dma_start(out=bconst_sb, in_=bconst)

            bias_hi = consts.tile([b_shard, D], BF16, tag="bias_hi")
            bias_lo = consts.tile([b_shard, D], BF16, tag="bias_lo")
            # batch 0's bias as a broadcast [128, D] f32 tile (late-bias path)
            bc0 = consts.tile([128, D], F32, tag="bc0")

            def emit_bias_computation():
                """s reduces + fp16 matvec + hi/lo split + batch-0 broadcast.

                Emitted at the end of rb3's block: by then the h stream
                (SWDGE) has fed the DVE reduces, and the PE reaches these
                matmuls at ~60us with sT16 long ready.
                """
                sT = consts.tile([128, n_dc, b_shard], F32, tag="sT", name="sT")
                gb = max(1, min(4, b_shard))
                for dc in range(n_dc):
                    for g in range(b_shard // gb):
                        cols = slice(g * gb * L, (g + 1) * gb * L)
                        htile = hpool.tile([128, gb * L], F16, tag="h", name="htile")
                        nc.gpsimd.dma_start(out=htile, in_=h_t16[dc, :, cols])
                        hview = htile.rearrange("p (b l) -> p b l", l=L)
                        nc.vector.reduce_sum(
                            sT[:, dc, g * gb:(g + 1) * gb], hview, axis=AX.X
                        )
                sT16 = consts.tile([128, n_dc, b_shard], F16, tag="sT16", name="sT16")
                with nc.allow_low_precision(reason="s matvec runs in fp16 on PE"):
                    nc.vector.tensor_copy(out=sT16, in_=sT)

                bias_ps = accpool.tile([b_shard, D], F32, tag="acc", name="bias_ps")
                n_wc = D // 256
                for wc in range(n_wc):
                    sl = slice(wc * 256, (wc + 1) * 256)
                    wschunk = wspool.tile(
                        [128, n_dc, 256], F16, tag="ws", name="wschunk"
                    )
                    nc.gpsimd.dma_start(
                        out=wschunk,
                        in_=ws_t.rearrange("dc p e -> p dc e")[:, :, sl],
                    )
                    for dc in range(n_dc):
                        nc.tensor.matmul(
                            bias_ps[:, sl],
                            sT16[:, dc, :],
                            wschunk[:, dc, :],
                            start=(dc == 0),
                            stop=False,
                        )
                    nc.tensor.matmul(
                        bias_ps[:, sl], ones1, bconst_sb[:, sl], start=False, stop=True
                    )

                # bf16 hi/lo split (bias_hi + bias_lo == bias to ~2e-4)
                nc.vector.tensor_copy(out=bias_hi, in_=bias_ps)
                bias_hi32 = small32.tile(
                    [b_shard, D], F32, tag="small32f", name="bias_hi32"
                )
                nc.vector.tensor_copy(out=bias_hi32, in_=bias_hi)
                nc.vector.tensor_tensor(
                    out=bias_lo, in0=bias_ps, in1=bias_hi32, op=ALU.subtract
                )

                # batch 0 broadcast bias via rank-2 ones-matmul -> f32 SBUF
                b2c = make_bias2_cur(0)
                for half in range(2):
                    bc_ps = zpool.tile([128, 1024], F32, tag="z", name="bc_ps")
                    for ec2 in range(2):
                        sl = slice(ec2 * 512, (ec2 + 1) * 512)
                        gsl = slice(half * 1024 + ec2 * 512, half * 1024 + (ec2 + 1) * 512)
                        nc.tensor.matmul(
                            bc_ps[:, sl], ones2, b2c[:, gsl], start=True, stop=True
                        )
                    nc.scalar.copy(
                        out=bc0[:, half * 1024:(half + 1) * 1024], in_=bc_ps
                    )
                # prefetch the next batches' bias tiles so rb4/rb8 never wait
                for nb in (1, 2):
                    if nb < b_shard:
                        b2cur[nb] = make_bias2_cur(nb)

            def make_bias2_cur(b):
                """Per-batch [2, D] (hi; lo) tile for the rank-2 PSUM fold."""
                b2c = b2pool.tile([2, D], BF16, tag="b2c", name=f"b2c_{b}")
                nc.gpsimd.dma_start(out=b2c[0:1], in_=bias_hi[b:b + 1])
                nc.gpsimd.dma_start(out=b2c[1:2], in_=bias_lo[b:b + 1])
                return b2c

            # ---- helpers ------------------------------------------------
            acc_box = {}
            acc_emitted = [0]

            def emit_acc(rb_, ww_, p_):
                if "acc" not in acc_box:
                    acc_box["acc"] = accpool.tile(
                        [b_shard, D], F32, tag="acc", name="acc_ps"
                    )
                acc_ps = acc_box["acc"]
                acc_emitted[0] += 1
                first = acc_emitted[0] == 1
                last = acc_emitted[0] == n_rb
                for ec in range(n_ec):
                    sl = slice(ec * 512, (ec + 1) * 512)
                    nc.tensor.matmul(
                        acc_ps[:, sl], ww_, p_[:, sl], start=first, stop=last
                    )

            def emit_z_a_mms(z, a_sb, eh, close):
                for kc in range(n_kc):
                    for ec2 in range(2):
                        ec = eh * 2 + ec2
                        nc.tensor.matmul(
                            z[:, ec2 * 512:(ec2 + 1) * 512],
                            a_sb[:, kc, :],
                            w_slice(kc, slice(ec * 512, (ec + 1) * 512)),
                            start=(kc == 0),
                            stop=(close and kc == n_kc - 1),
                        )

            def emit_softmax_tail(b, rb, p_tile, s2):
                s_sum = spool.tile([128, 1], F32, tag="s_sum")
                nc.vector.tensor_tensor(
                    out=s_sum, in0=s2[:, 0:1], in1=s2[:, 1:2], op=ALU.add
                )
                w_rcp = spool.tile([128, 1], F32, tag="w_rcp")
                nc.vector.reciprocal(out=w_rcp, in_=s_sum)
                ww = spool.tile([128, b_shard], BF16, tag="ww")
                nc.vector.memset(ww, 0.0)
                nc.vector.tensor_copy(out=ww[:, b:b + 1], in_=w_rcp)
                return ww

            def exp_half(p_tile, t_in, rb, eh, s2):
                nc.scalar.activation(
                    p_tile[:, eh * 1024:(eh + 1) * 1024],
                    t_in,
                    AF.Exp,
                    scale=mask_sb[:, rb:rb + 1],
                    accum_out=s2[:, eh:eh + 1],
                )

            stash = []
            deferred = []

            def finish_one_stash():
                """Produce p for ONE stashed row block (DVE add + ACT chain).
                Interleaved one-per-rb so the ACT queue never backs up behind
                the stash work (in-order engine streams); the acc matmuls
                drain through the deferred queue."""
                rb_, b_, z_sb = stash.pop(0)
                p_tile = pstash.tile([128, D], BF16, tag="pst", name=f"pst{rb_}")
                s2 = spool.tile([128, 2], F32, tag="s2", name=f"s2st{rb_}")
                for eh in range(2):
                    hs = slice(eh * 1024, (eh + 1) * 1024)
                    ta = tpool.tile([128, 1024], BF16, tag="t", name=f"ta{rb_}{eh}")
                    nc.vector.tensor_tensor(
                        out=ta, in0=z_sb[:, hs], in1=bc0[:, hs], op=ALU.add
                    )
                    t_tile = tpool.tile(
                        [128, 1024], BF16, tag="t", name=f"t{rb_}{eh}"
                    )
                    nc.scalar.activation(t_tile, ta, AF.Tanh)
                    exp_half(p_tile, t_tile, rb_, eh, s2)
                ww = emit_softmax_tail(b_, rb_, p_tile, s2)
                deferred.append((rb_, ww, p_tile))

            # ---- main loop ----------------------------------------------
            b2cur = {}
            for b in range(b_shard):
                for rbl in range(rb_per_batch):
                    rb = b * rb_per_batch + rbl
                    if stash and rb >= n_stash:
                        with nc.named_scope("stashfin"):
                            finish_one_stash()
                    if rbl == 0 and b >= 1 and b not in b2cur:
                        b2cur[b] = make_bias2_cur(b)

                    if rb in a_sb_pre:
                        a_sb = a_sb_pre.pop(rb)
                    else:
                        a_sb = apool.tile([128, n_kc, 128], BF16, tag="a_sb")
                        nc.sync.dma_start(out=a_sb, in_=a_t[rb])

                    if rb < n_stash:
                        # late-bias path: close groups without bias, stash z
                        z_sb = zstash.tile([128, D], F16, tag="zst", name=f"zst{rb}")
                        for eh in range(2):
                            z = zpool.tile([128, 1024], F32, tag="z")
                            emit_z_a_mms(z, a_sb, eh, close=True)
                            nc.scalar.copy(
                                out=z_sb[:, eh * 1024:(eh + 1) * 1024], in_=z
                            )
                        stash.append((rb, b, z_sb))
                        if rb == n_stash - 1:
                            with nc.named_scope("biascomp"):
                                emit_bias_computation()
                        continue

                    p_tile = ppool.tile([128, D], BF16, tag="p")
                    s2 = spool.tile([128, 2], F32, tag="s2")
                    for eh in range(2):
                        z = zpool.tile([128, 1024], F32, tag="z")
                        emit_z_a_mms(z, a_sb, eh, close=False)
                        for ec2 in range(2):
                            ec = eh * 2 + ec2
                            nc.tensor.matmul(
                                z[:, ec2 * 512:(ec2 + 1) * 512],
                                ones2,
                                b2cur[b][:, ec * 512:(ec + 1) * 512],
                                start=False,
                                stop=True,
                            )
                        t_tile = tpool.tile([128, 1024], BF16, tag="t")
                        nc.scalar.activation(t_tile, z, AF.Tanh)
                        exp_half(p_tile, t_tile, rb, eh, s2)

                    ww = emit_softmax_tail(b, rb, p_tile, s2)
                    deferred.append((rb, ww, p_tile))
                    while len(deferred) > 2:
                        emit_acc(*deferred.pop(0))

            while stash:
                finish_one_stash()
            for item in deferred:
                emit_acc(*item)

            # ---- finalize: out = acc * trig_a ---------------------------
            trig_sb = small32.tile([b_shard, D], F32, tag="small32f")
            nc.sync.dma_start(out=trig_sb, in_=trig)
            out_sb = small32.tile([b_shard, D], F32, tag="small32f")
            nc.vector.tensor_tensor(
                out=out_sb, in0=acc_box["acc"], in1=trig_sb, op=ALU.mult
            )
            nc.sync.dma_start(out=out, in_=out_sb)

    nc.compile()
    return nc


def prep_core_inputs(h_c, x_c, trigger_c, mask_c, w_t, ws_t, bconst):
    """Host-side data prep for one core's shard (b_shard batches)."""
    b_shard = h_c.shape[0]
    rows = b_shard * L
    n_rb = rows // 128
    n_dc = H // 128

    a_core = np.concatenate(
        [h_c.reshape(rows, H), x_c.reshape(rows, X)], axis=1
    )  # [rows, D] f32

    a_t = np.ascontiguousarray(
        a_core.reshape(n_rb, 128, D // 128, 128).transpose(0, 3, 2, 1)
    ).astype(ml_dtypes.bfloat16)

    h_t16 = np.ascontiguousarray(h_c.reshape(rows, H).T).reshape(n_dc, 128, rows)
    h_t16 = np.ascontiguousarray(h_t16, dtype=np.float16)

    maskf = np.ascontiguousarray(
        mask_c.reshape(rows).reshape(n_rb, 128).T
    ).astype(np.float32)

    tr = np.asarray(trigger_c).reshape(b_shard).astype(np.int64)
    trig = a_core[np.arange(b_shard) * L + tr].astype(np.float32)

    return {
        "a_t": a_t,
        "w_t": w_t,
        "h_t16": h_t16,
        "ws_t": ws_t,
        "bconst": bconst,
        "maskf": maskf,
        "trig": np.ascontiguousarray(trig),
    }


_PROGRAM_CACHE = {}


def build_in_maps(h_state, x, W_a, b_a, W_s, b_s, trigger, mask):
    h_state = np.asarray(h_state, dtype=np.float32)
    x = np.asarray(x, dtype=np.float32)
    W_a = np.asarray(W_a, dtype=np.float32)
    b_a = np.asarray(b_a, dtype=np.float32)
    W_s = np.asarray(W_s, dtype=np.float32)
    b_s = np.asarray(b_s, dtype=np.float32)
    trigger = np.asarray(trigger)
    mask = np.asarray(mask)

    # replicated weights
    w_t = np.ascontiguousarray(W_a.T).astype(ml_dtypes.bfloat16).reshape(
        D // 128, 128, D
    )
    ws_t = np.ascontiguousarray(W_s.T, dtype=np.float16).reshape(H // 128, 128, D)
    bconst = np.ascontiguousarray((b_a + b_s).reshape(1, D), dtype=np.float16)

    in_maps = []
    for c in range(N_CORES):
        bs = slice(c * B_SHARD, (c + 1) * B_SHARD)
        in_maps.append(
            prep_core_inputs(
                h_state[bs], x[bs], trigger[bs], mask[bs], w_t, ws_t, bconst
            )
        )
    return in_maps


def kernel(h_state, x, W_a, b_a, W_s, b_s, trigger, mask):
    global LAST_RESULT
    in_maps = build_in_maps(h_state, x, W_a, b_a, W_s, b_s, trigger, mask)

    if B_SHARD not in _PROGRAM_CACHE:
        _PROGRAM_CACHE[B_SHARD] = build_program(B_SHARD)
    nc = _PROGRAM_CACHE[B_SHARD]

    import os
    trace = bool(int(os.environ.get("KERNEL_TRACE", "0")))
    res = run_bass_kernel_spmd(nc, in_maps, core_ids=list(range(N_CORES)), trace=trace)
    LAST_RESULT = res

    out = np.empty((B, D), dtype=np.float32)
    for c in range(N_CORES):
        out[c * B_SHARD:(c + 1) * B_SHARD] = res.results[c]["out"]
    return out


# revision 42
# speedup vs baseline: 1.0161x; 1.0161x over previous
"""Trainium2 Bass kernel for nn_Decoder_22273700397266 (sparse_attention).

Reference math (B=64, L=512, H=X=1024, D=2048):
    a   = concat(h_state, x, -1)                         # [B, L, D]
    s   = h_state.sum(axis=1)                            # [B, H]
    et  = tanh(a @ W_a.T + b_a + (s @ W_s.T + b_s)[:,None,:])
    et  = where(mask==0, -1e9, et)
    attn = softmax(et, axis=-1)                          # over D
    out = (a[b, trigger[b]] * attn).sum(axis=1)          # [B, D]

Sharding: data-parallel over batch across 8 NeuronCores (8 batches/core);
W_a / W_s replicated.

Per-core kernel design:
  * main matmul z = a @ W_a.T in bf16: rows on PSUM partitions, features on
    the free dim (the softmax axis), K-chunked over 16x128; W_a^T resident in
    SBUF (4 sub-tiles so matmuls start as soon as the first DMA quarter lands).
  * per-batch bias vector bias_b = b_a + b_s + W_s @ s_b:
      - s_b = sum_l h[b,l,:] via DVE reduces over an fp16 h stream (SWDGE
        queue so it never blocks the main DMA stream),
      - fp16 matvec on PE, bf16 hi+lo split (keeps |bias|~13 to ~2e-4),
      - folded into the z PSUM accumulation with one rank-2 ones-matmul.
  * the first batch's 4 row blocks cannot wait for the bias (it needs the
    whole h stream): their z groups close WITHOUT bias, z is stashed to SBUF
    (fp16), and once the bias exists it is applied as a broadcast [128, D]
    tile (ones-matmul into PSUM -> f32 SBUF) with a DVE add. This keeps the
    PE dense from ~8us onward instead of idling ~90us on the bias path.
  * tanh in [-1,1] -> softmax needs no max subtraction; masking folds into
    the exp as p = exp(mask * t): masked rows give exp(0)=1 everywhere =
    exactly the uniform softmax the reference produces for -1e9 rows.
  * denominator S = sum_e p comes free from the exp op's accum_out.
  * sum_l attn is a second matmul: acc[8, e] += w_wide.T @ p with
    w_wide[:, b] = 1/S (zero elsewhere), accumulated over all 32 row blocks.
  * out = acc * trig_a (host-gathered trigger rows), one [8, 2048] DMA out.
"""

import numpy as np
import ml_dtypes

import concourse.bass as bass
import concourse.bacc as bacc
import concourse.mybir as mybir
import concourse.tile as tile
from concourse.bass_utils import run_bass_kernel_spmd

F32 = mybir.dt.float32
F16 = mybir.dt.float16
BF16 = mybir.dt.bfloat16
AF = mybir.ActivationFunctionType
ALU = mybir.AluOpType
AX = mybir.AxisListType

B, L, H, X = 64, 512, 1024, 1024
D = H + X  # 2048
N_CORES = 8
B_SHARD = B // N_CORES  # 8

# Diagnostics stashed by kernel() for test harnesses.
LAST_RESULT = None


def build_program(b_shard: int = B_SHARD):
    """Build the per-core Bass program. Same program runs SPMD on all cores."""
    rows = b_shard * L              # 4096
    n_rb = rows // 128              # 32 row blocks of 128 rows
    rb_per_batch = L // 128         # 4
    n_kc = D // 128                 # 16 contraction chunks for the main matmul
    n_dc = H // 128                 # 8 contraction chunks for the s matvec
    n_ec = D // 512                 # 4 feature chunks of 512

    n_stash = rb_per_batch          # batch 0's row blocks take the late-bias path

    nc = bacc.Bacc("TRN2", target_bir_lowering=False, debug=False)

    a_t = nc.dram_tensor("a_t", [n_rb, 128, n_kc, 128], BF16, kind="ExternalInput").ap()
    w_t = nc.dram_tensor("w_t", [n_kc, 128, D], BF16, kind="ExternalInput").ap()
    h_t16 = nc.dram_tensor("h_t16", [n_dc, 128, rows], F16, kind="ExternalInput").ap()
    ws_t = nc.dram_tensor("ws_t", [n_dc, 128, D], F16, kind="ExternalInput").ap()
    bconst = nc.dram_tensor("bconst", [1, D], F16, kind="ExternalInput").ap()
    maskf = nc.dram_tensor("maskf", [128, n_rb], F32, kind="ExternalInput").ap()
    trig = nc.dram_tensor("trig", [b_shard, D], F32, kind="ExternalInput").ap()
    out = nc.dram_tensor("out", [b_shard, D], F32, kind="ExternalOutput").ap()

    with tile.TileContext(nc) as tc:
        with (
            tc.tile_pool(name="consts", bufs=1) as consts,
            tc.tile_pool(name="hpool", bufs=3) as hpool,
            tc.tile_pool(name="wspool", bufs=1) as wspool,
            tc.tile_pool(name="b2pool", bufs=3) as b2pool,
            tc.tile_pool(name="small32", bufs=1) as small32,
            tc.tile_pool(name="apool", bufs=3) as apool,
            tc.tile_pool(name="ppool", bufs=3) as ppool,
            tc.tile_pool(name="pstash", bufs=3) as pstash,
            tc.tile_pool(name="zstash", bufs=n_stash) as zstash,
            tc.tile_pool(name="tpool", bufs=3) as tpool,
            tc.tile_pool(name="spool", bufs=6) as spool,
            tc.tile_pool(name="zpool", bufs=2, space="PSUM") as zpool,
            tc.tile_pool(name="accpool", bufs=1, space="PSUM") as accpool,
        ):
            # ---- resident constants -------------------------------------
            mask_sb = consts.tile([128, n_rb], F32, tag="mask_sb")
            nc.sync.dma_start(out=mask_sb, in_=maskf)

            # W_a^T in 8 sub-tiles, all on the sync queue, interleaved with
            # the first a tiles: the PE's first matmuls start as soon as the
            # first W chunk lands (~13us incl the ~8us DMA startup) and are
            # then paced by the W stream until ~30us.
            kq = n_kc // 8
            w_sbs = []
            w_aps = [
                w_t.rearrange("kc p e -> p kc e")[:, q * kq:(q + 1) * kq, :]
                for q in range(8)
            ]
            a_sb_pre = {}

            def pre_a(rb):
                a_sb_pre[rb] = apool.tile(
                    [128, n_kc, 128], BF16, tag="a_sb", name=f"a_sb{rb}"
                )
                nc.sync.dma_start(out=a_sb_pre[rb], in_=a_t[rb])

            pre_a(0)
            for q in range(8):
                w_q = consts.tile([128, kq, D], BF16, tag=f"w_sb{q}", name=f"w_sb{q}")
                nc.sync.dma_start(out=w_q, in_=w_aps[q])
                w_sbs.append(w_q)
                if q == 1:
                    pre_a(1)
                if q == 3:
                    pre_a(2)

            def w_slice(kc, sl):
                return w_sbs[kc // kq][:, kc % kq, sl]

            # tiny DMA depending on the last W chunk: sequences the gpsimd
            # queue so the h stream only starts once the W/a prefetch has had
            # the full HBM bandwidth (h isn't consumed until ~55us anyway)
            hgate = consts.tile([1, 2], BF16, tag="hgate")
            nc.gpsimd.dma_start(out=hgate, in_=w_sbs[7][0:1, 0, 0:2])

            ones2 = consts.tile([2, 128], BF16, tag="ones2")
            nc.vector.memset(ones2, 1.0)
            ones1 = consts.tile([1, b_shard], F16, tag="ones1")
            nc.vector.memset(ones1, 1.0)

            bconst_sb = consts.tile([1, D], F16, tag="bconst_sb")
            nc.gpsimd.dma_start(out=bconst_sb, in_=bconst)

            bias_hi = consts.tile([b_shard, D], BF16, tag="bias_hi")
            bias_lo = consts.tile([b_shard, D], BF16, tag="bias_lo")
            # batch 0's bias as a broadcast [128, D] f32 tile (late-bias path)
            bc0 = consts.tile([128, D], F32, tag="bc0")

            def emit_bias_computation():
                """s reduces + fp16 matvec + hi/lo split + batch-0 broadcast.

                Emitted at the end of rb3's block: by then the h stream
                (SWDGE) has fed the DVE reduces, and the PE reaches these
                matmuls at ~60us with sT16 long ready.
                """
                sT = consts.tile([128, n_dc, b_shard], F32, tag="sT", name="sT")
                gb = max(1, min(4, b_shard))
                for dc in range(n_dc):
                    for g in range(b_shard // gb):
                        cols = slice(g * gb * L, (g + 1) * gb * L)
                        htile = hpool.tile([128, gb * L], F16, tag="h", name="htile")
                        nc.gpsimd.dma_start(out=htile, in_=h_t16[dc, :, cols])
                        hview = htile.rearrange("p (b l) -> p b l", l=L)
                        nc.vector.reduce_sum(
                            sT[:, dc, g * gb:(g + 1) * gb], hview, axis=AX.X
                        )
                sT16 = consts.tile([128, n_dc, b_shard], F16, tag="sT16", name="sT16")
                with nc.allow_low_precision(reason="s matvec runs in fp16 on PE"):
                    nc.vector.tensor_copy(out=sT16, in_=sT)

                # whole W_s^T resident (one 4.2MB DMA, behind h on the gpsimd
                # queue) so the matvec below runs at PE speed, not DMA-paced
                ws_sb = wspool.tile([128, n_dc, D], F16, tag="ws", name="ws_sb")
                nc.gpsimd.dma_start(
                    out=ws_sb, in_=ws_t.rearrange("dc p e -> p dc e")
                )

                bias_ps = accpool.tile([b_shard, D], F32, tag="acc", name="bias_ps")
                for ec in range(n_ec):
                    sl = slice(ec * 512, (ec + 1) * 512)
                    for dc in range(n_dc):
                        nc.tensor.matmul(
                            bias_ps[:, sl],
                            sT16[:, dc, :],
                            ws_sb[:, dc, sl],
                            start=(dc == 0),
                            stop=False,
                        )
                    nc.tensor.matmul(
                        bias_ps[:, sl], ones1, bconst_sb[:, sl], start=False, stop=True
                    )

                # bf16 hi/lo split (bias_hi + bias_lo == bias to ~2e-4)
                nc.vector.tensor_copy(out=bias_hi, in_=bias_ps)
                nc.vector.tensor_tensor(
                    out=bias_lo, in0=bias_ps, in1=bias_hi, op=ALU.subtract
                )

                # batch 0 broadcast bias via rank-2 ones-matmul -> f32 SBUF
                b2c = make_bias2_cur(0)
                for half in range(2):
                    bc_ps = zpool.tile([128, 1024], F32, tag="z", name="bc_ps")
                    for ec2 in range(2):
                        sl = slice(ec2 * 512, (ec2 + 1) * 512)
                        gsl = slice(half * 1024 + ec2 * 512, half * 1024 + (ec2 + 1) * 512)
                        nc.tensor.matmul(
                            bc_ps[:, sl], ones2, b2c[:, gsl], start=True, stop=True
                        )
                    nc.scalar.copy(
                        out=bc0[:, half * 1024:(half + 1) * 1024], in_=bc_ps
                    )
                # prefetch the next batches' bias tiles so rb4/rb8 never wait
                for nb in (1, 2):
                    if nb < b_shard:
                        b2cur[nb] = make_bias2_cur(nb)

            def make_bias2_cur(b):
                """Per-batch [2, D] (hi; lo) tile for the rank-2 PSUM fold."""
                b2c = b2pool.tile([2, D], BF16, tag="b2c", name=f"b2c_{b}")
                nc.gpsimd.dma_start(out=b2c[0:1], in_=bias_hi[b:b + 1])
                nc.gpsimd.dma_start(out=b2c[1:2], in_=bias_lo[b:b + 1])
                return b2c

            # ---- helpers ------------------------------------------------
            acc_box = {}
            acc_emitted = [0]

            def emit_acc(rb_, ww_, p_):
                if "acc" not in acc_box:
                    acc_box["acc"] = accpool.tile(
                        [b_shard, D], F32, tag="acc", name="acc_ps"
                    )
                acc_ps = acc_box["acc"]
                acc_emitted[0] += 1
                first = acc_emitted[0] == 1
                last = acc_emitted[0] == n_rb
                for ec in range(n_ec):
                    sl = slice(ec * 512, (ec + 1) * 512)
                    nc.tensor.matmul(
                        acc_ps[:, sl], ww_, p_[:, sl], start=first, stop=last
                    )

            def emit_z_a_mms(z, a_sb, eh, close):
                for kc in range(n_kc):
                    for ec2 in range(2):
                        ec = eh * 2 + ec2
                        nc.tensor.matmul(
                            z[:, ec2 * 512:(ec2 + 1) * 512],
                            a_sb[:, kc, :],
                            w_slice(kc, slice(ec * 512, (ec + 1) * 512)),
                            start=(kc == 0),
                            stop=(close and kc == n_kc - 1),
                        )

            def emit_softmax_tail(b, rb, p_tile, s2):
                s_sum = spool.tile([128, 1], F32, tag="s_sum")
                nc.vector.tensor_tensor(
                    out=s_sum, in0=s2[:, 0:1], in1=s2[:, 1:2], op=ALU.add
                )
                w_rcp = spool.tile([128, 1], F32, tag="w_rcp")
                nc.vector.reciprocal(out=w_rcp, in_=s_sum)
                ww = spool.tile([128, b_shard], BF16, tag="ww")
                nc.vector.memset(ww, 0.0)
                nc.vector.tensor_copy(out=ww[:, b:b + 1], in_=w_rcp)
                return ww

            def exp_half(p_tile, t_in, rb, eh, s2):
                nc.scalar.activation(
                    p_tile[:, eh * 1024:(eh + 1) * 1024],
                    t_in,
                    AF.Exp,
                    scale=mask_sb[:, rb:rb + 1],
                    accum_out=s2[:, eh:eh + 1],
                )

            stash = []
            deferred = []

            def finish_one_stash():
                """Produce p for ONE stashed row block (DVE add + ACT chain).
                Interleaved one-per-rb so the ACT queue never backs up behind
                the stash work (in-order engine streams); the acc matmuls
                drain through the deferred queue."""
                rb_, b_, z_sb = stash.pop(0)
                p_tile = pstash.tile([128, D], BF16, tag="pst", name=f"pst{rb_}")
                s2 = spool.tile([128, 2], F32, tag="s2", name=f"s2st{rb_}")
                for eh in range(2):
                    hs = slice(eh * 1024, (eh + 1) * 1024)
                    ta = tpool.tile([128, 1024], BF16, tag="t", name=f"ta{rb_}{eh}")
                    nc.vector.tensor_tensor(
                        out=ta, in0=z_sb[:, hs], in1=bc0[:, hs], op=ALU.add
                    )
                    t_tile = tpool.tile(
                        [128, 1024], BF16, tag="t", name=f"t{rb_}{eh}"
                    )
                    nc.scalar.activation(t_tile, ta, AF.Tanh)
                    exp_half(p_tile, t_tile, rb_, eh, s2)
                ww = emit_softmax_tail(b_, rb_, p_tile, s2)
                deferred.append((rb_, ww, p_tile))

            # ---- main loop ----------------------------------------------
            b2cur = {}
            for b in range(b_shard):
                for rbl in range(rb_per_batch):
                    rb = b * rb_per_batch + rbl
                    if stash and rb >= n_stash:
                        with nc.named_scope("stashfin"):
                            finish_one_stash()
                    if rbl == 0 and b >= 1 and b not in b2cur:
                        b2cur[b] = make_bias2_cur(b)

                    if rb in a_sb_pre:
                        a_sb = a_sb_pre.pop(rb)
                    else:
                        a_sb = apool.tile([128, n_kc, 128], BF16, tag="a_sb")
                        nc.sync.dma_start(out=a_sb, in_=a_t[rb])

                    if rb < n_stash:
                        # late-bias path: close groups without bias, stash z
                        z_sb = zstash.tile([128, D], F16, tag="zst", name=f"zst{rb}")
                        for eh in range(2):
                            z = zpool.tile([128, 1024], F32, tag="z")
                            emit_z_a_mms(z, a_sb, eh, close=True)
                            nc.scalar.copy(
                                out=z_sb[:, eh * 1024:(eh + 1) * 1024], in_=z
                            )
                        stash.append((rb, b, z_sb))
                        if rb == n_stash - 2:
                            with nc.named_scope("biascomp"):
                                emit_bias_computation()
                        continue

                    p_tile = ppool.tile([128, D], BF16, tag="p")
                    s2 = spool.tile([128, 2], F32, tag="s2")
                    for eh in range(2):
                        z = zpool.tile([128, 1024], F32, tag="z")
                        emit_z_a_mms(z, a_sb, eh, close=False)
                        for ec2 in range(2):
                            ec = eh * 2 + ec2
                            nc.tensor.matmul(
                                z[:, ec2 * 512:(ec2 + 1) * 512],
                                ones2,
                                b2cur[b][:, ec * 512:(ec + 1) * 512],
                                start=False,
                                stop=True,
                            )
                        t_tile = tpool.tile([128, 1024], BF16, tag="t")
                        nc.scalar.activation(t_tile, z, AF.Tanh)
                        exp_half(p_tile, t_tile, rb, eh, s2)

                    ww = emit_softmax_tail(b, rb, p_tile, s2)
                    deferred.append((rb, ww, p_tile))
                    while len(deferred) > 2:
                        emit_acc(*deferred.pop(0))

            while stash:
                finish_one_stash()
            for item in deferred:
                emit_acc(*item)

            # ---- finalize: out = acc * trig_a ---------------------------
            trig_sb = small32.tile([b_shard, D], F32, tag="small32f")
            nc.sync.dma_start(out=trig_sb, in_=trig)
            nc.vector.tensor_tensor(
                out=trig_sb, in0=acc_box["acc"], in1=trig_sb, op=ALU.mult
            )
            nc.sync.dma_start(out=out, in_=trig_sb)

    nc.compile()
    return nc


def prep_core_inputs(h_c, x_c, trigger_c, mask_c, w_t, ws_t, bconst):
    """Host-side data prep for one core's shard (b_shard batches)."""
    b_shard = h_c.shape[0]
    rows = b_shard * L
    n_rb = rows // 128
    n_dc = H // 128

    a_core = np.concatenate(
        [h_c.reshape(rows, H), x_c.reshape(rows, X)], axis=1
    )  # [rows, D] f32

    a_t = np.ascontiguousarray(
        a_core.reshape(n_rb, 128, D // 128, 128).transpose(0, 3, 2, 1)
    ).astype(ml_dtypes.bfloat16)

    h_t16 = np.ascontiguousarray(h_c.reshape(rows, H).T).reshape(n_dc, 128, rows)
    h_t16 = np.ascontiguousarray(h_t16, dtype=np.float16)

    maskf = np.ascontiguousarray(
        mask_c.reshape(rows).reshape(n_rb, 128).T
    ).astype(np.float32)

    tr = np.asarray(trigger_c).reshape(b_shard).astype(np.int64)
    trig = a_core[np.arange(b_shard) * L + tr].astype(np.float32)

    return {
        "a_t": a_t,
        "w_t": w_t,
        "h_t16": h_t16,
        "ws_t": ws_t,
        "bconst": bconst,
        "maskf": maskf,
        "trig": np.ascontiguousarray(trig),
    }


_PROGRAM_CACHE = {}


def build_in_maps(h_state, x, W_a, b_a, W_s, b_s, trigger, mask):
    h_state = np.asarray(h_state, dtype=np.float32)
    x = np.asarray(x, dtype=np.float32)
    W_a = np.asarray(W_a, dtype=np.float32)
    b_a = np.asarray(b_a, dtype=np.float32)
    W_s = np.asarray(W_s, dtype=np.float32)
    b_s = np.asarray(b_s, dtype=np.float32)
    trigger = np.asarray(trigger)
    mask = np.asarray(mask)

    # replicated weights
    w_t = np.ascontiguousarray(W_a.T).astype(ml_dtypes.bfloat16).reshape(
        D // 128, 128, D
    )
    ws_t = np.ascontiguousarray(W_s.T, dtype=np.float16).reshape(H // 128, 128, D)
    bconst = np.ascontiguousarray((b_a + b_s).reshape(1, D), dtype=np.float16)

    in_maps = []
    for c in range(N_CORES):
        bs = slice(c * B_SHARD, (c + 1) * B_SHARD)
        in_maps.append(
            prep_core_inputs(
                h_state[bs], x[bs], trigger[bs], mask[bs], w_t, ws_t, bconst
            )
        )
    return in_maps


def kernel(h_state, x, W_a, b_a, W_s, b_s, trigger, mask):
    global LAST_RESULT
    in_maps = build_in_maps(h_state, x, W_a, b_a, W_s, b_s, trigger, mask)

    if B_SHARD not in _PROGRAM_CACHE:
        _PROGRAM_CACHE[B_SHARD] = build_program(B_SHARD)
    nc = _PROGRAM_CACHE[B_SHARD]

    import os
    trace = bool(int(os.environ.get("KERNEL_TRACE", "0")))
    res = run_bass_kernel_spmd(nc, in_maps, core_ids=list(range(N_CORES)), trace=trace)
    LAST_RESULT = res

    out = np.empty((B, D), dtype=np.float32)
    for c in range(N_CORES):
        out[c * B_SHARD:(c + 1) * B_SHARD] = res.results[c]["out"]
    return out


# revision 43
# speedup vs baseline: 1.0236x; 1.0074x over previous
"""Trainium2 Bass kernel for nn_Decoder_22273700397266 (sparse_attention).

Reference math (B=64, L=512, H=X=1024, D=2048):
    a   = concat(h_state, x, -1)                         # [B, L, D]
    s   = h_state.sum(axis=1)                            # [B, H]
    et  = tanh(a @ W_a.T + b_a + (s @ W_s.T + b_s)[:,None,:])
    et  = where(mask==0, -1e9, et)
    attn = softmax(et, axis=-1)                          # over D
    out = (a[b, trigger[b]] * attn).sum(axis=1)          # [B, D]

Sharding: data-parallel over batch across 8 NeuronCores (8 batches/core);
W_a / W_s replicated.

Per-core kernel design:
  * main matmul z = a @ W_a.T in bf16: rows on PSUM partitions, features on
    the free dim (the softmax axis), K-chunked over 16x128; W_a^T resident in
    SBUF (4 sub-tiles so matmuls start as soon as the first DMA quarter lands).
  * per-batch bias vector bias_b = b_a + b_s + W_s @ s_b:
      - s_b = sum_l h[b,l,:] via DVE reduces over an fp16 h stream (SWDGE
        queue so it never blocks the main DMA stream),
      - fp16 matvec on PE, bf16 hi+lo split (keeps |bias|~13 to ~2e-4),
      - folded into the z PSUM accumulation with one rank-2 ones-matmul.
  * the first batch's 4 row blocks cannot wait for the bias (it needs the
    whole h stream): their z groups close WITHOUT bias, z is stashed to SBUF
    (fp16), and once the bias exists it is applied as a broadcast [128, D]
    tile (ones-matmul into PSUM -> f32 SBUF) with a DVE add. This keeps the
    PE dense from ~8us onward instead of idling ~90us on the bias path.
  * tanh in [-1,1] -> softmax needs no max subtraction; masking folds into
    the exp as p = exp(mask * t): masked rows give exp(0)=1 everywhere =
    exactly the uniform softmax the reference produces for -1e9 rows.
  * denominator S = sum_e p comes free from the exp op's accum_out.
  * sum_l attn is a second matmul: acc[8, e] += w_wide.T @ p with
    w_wide[:, b] = 1/S (zero elsewhere), accumulated over all 32 row blocks.
  * out = acc * trig_a (host-gathered trigger rows), one [8, 2048] DMA out.
"""

import numpy as np
import ml_dtypes

import concourse.bass as bass
import concourse.bacc as bacc
import concourse.mybir as mybir
import concourse.tile as tile
from concourse.bass_utils import run_bass_kernel_spmd

F32 = mybir.dt.float32
F16 = mybir.dt.float16
BF16 = mybir.dt.bfloat16
AF = mybir.ActivationFunctionType
ALU = mybir.AluOpType
AX = mybir.AxisListType

B, L, H, X = 64, 512, 1024, 1024
D = H + X  # 2048
N_CORES = 8
B_SHARD = B // N_CORES  # 8

# Diagnostics stashed by kernel() for test harnesses.
LAST_RESULT = None


def build_program(b_shard: int = B_SHARD):
    """Build the per-core Bass program. Same program runs SPMD on all cores."""
    rows = b_shard * L              # 4096
    n_rb = rows // 128              # 32 row blocks of 128 rows
    rb_per_batch = L // 128         # 4
    n_kc = D // 128                 # 16 contraction chunks for the main matmul
    n_dc = H // 128                 # 8 contraction chunks for the s matvec
    n_ec = D // 512                 # 4 feature chunks of 512

    n_stash = rb_per_batch          # batch 0's row blocks take the late-bias path

    nc = bacc.Bacc("TRN2", target_bir_lowering=False, debug=False)

    a_t = nc.dram_tensor("a_t", [n_rb, 128, n_kc, 128], BF16, kind="ExternalInput").ap()
    w_t = nc.dram_tensor("w_t", [128, n_kc, D], BF16, kind="ExternalInput").ap()
    h_t16 = nc.dram_tensor("h_t16", [n_dc, 128, rows], F16, kind="ExternalInput").ap()
    ws_t = nc.dram_tensor("ws_t", [128, n_dc, D], F16, kind="ExternalInput").ap()
    bconst = nc.dram_tensor("bconst", [1, D], F16, kind="ExternalInput").ap()
    maskf = nc.dram_tensor("maskf", [128, n_rb], F32, kind="ExternalInput").ap()
    trig = nc.dram_tensor("trig", [b_shard, D], F32, kind="ExternalInput").ap()
    out = nc.dram_tensor("out", [b_shard, D], F32, kind="ExternalOutput").ap()

    with tile.TileContext(nc) as tc:
        with (
            tc.tile_pool(name="consts", bufs=1) as consts,
            tc.tile_pool(name="hpool", bufs=3) as hpool,
            tc.tile_pool(name="wspool", bufs=1) as wspool,
            tc.tile_pool(name="b2pool", bufs=3) as b2pool,
            tc.tile_pool(name="small32", bufs=1) as small32,
            tc.tile_pool(name="apool", bufs=3) as apool,
            tc.tile_pool(name="ppool", bufs=3) as ppool,
            tc.tile_pool(name="pstash", bufs=3) as pstash,
            tc.tile_pool(name="zstash", bufs=n_stash) as zstash,
            tc.tile_pool(name="tpool", bufs=3) as tpool,
            tc.tile_pool(name="spool", bufs=6) as spool,
            tc.tile_pool(name="zpool", bufs=2, space="PSUM") as zpool,
            tc.tile_pool(name="accpool", bufs=1, space="PSUM") as accpool,
        ):
            # ---- resident constants -------------------------------------
            mask_sb = consts.tile([128, n_rb], F32, tag="mask_sb")
            nc.sync.dma_start(out=mask_sb, in_=maskf)

            # W_a^T in 8 sub-tiles, all on the sync queue, interleaved with
            # the first a tiles: the PE's first matmuls start as soon as the
            # first W chunk lands (~13us incl the ~8us DMA startup) and are
            # then paced by the W stream until ~30us.
            kq = n_kc // 8
            w_sbs = []
            w_aps = [w_t[:, q * kq:(q + 1) * kq, :] for q in range(8)]
            a_sb_pre = {}

            def pre_a(rb):
                a_sb_pre[rb] = apool.tile(
                    [128, n_kc, 128], BF16, tag="a_sb", name=f"a_sb{rb}"
                )
                nc.sync.dma_start(out=a_sb_pre[rb], in_=a_t[rb])

            pre_a(0)
            for q in range(8):
                w_q = consts.tile([128, kq, D], BF16, tag=f"w_sb{q}", name=f"w_sb{q}")
                eng = nc.sync if q % 2 == 0 else nc.scalar
                eng.dma_start(out=w_q, in_=w_aps[q])
                w_sbs.append(w_q)
                if q == 1:
                    pre_a(1)
                if q == 3:
                    pre_a(2)

            def w_slice(kc, sl):
                return w_sbs[kc // kq][:, kc % kq, sl]

            # tiny DMA depending on the last W chunk: sequences the gpsimd
            # queue so the h stream only starts once the W/a prefetch has had
            # the full HBM bandwidth (h isn't consumed until ~55us anyway)
            hgate = consts.tile([1, 2], BF16, tag="hgate")
            nc.gpsimd.dma_start(out=hgate, in_=w_sbs[7][0:1, 0, 0:2])

            ones2 = consts.tile([2, 128], BF16, tag="ones2")
            nc.vector.memset(ones2, 1.0)
            ones1 = consts.tile([1, b_shard], F16, tag="ones1")
            nc.vector.memset(ones1, 1.0)

            bconst_sb = consts.tile([1, D], F16, tag="bconst_sb")
            nc.gpsimd.dma_start(out=bconst_sb, in_=bconst)

            bias_hi = consts.tile([b_shard, D], BF16, tag="bias_hi")
            bias_lo = consts.tile([b_shard, D], BF16, tag="bias_lo")
            # batch 0's bias as a broadcast [128, D] f32 tile (late-bias path)
            bc0 = consts.tile([128, D], F32, tag="bc0")

            def emit_bias_computation():
                """s reduces + fp16 matvec + hi/lo split + batch-0 broadcast.

                Emitted at the end of rb3's block: by then the h stream
                (SWDGE) has fed the DVE reduces, and the PE reaches these
                matmuls at ~60us with sT16 long ready.
                """
                sT = consts.tile([128, n_dc, b_shard], F32, tag="sT", name="sT")
                gb = max(1, min(4, b_shard))
                for dc in range(n_dc):
                    for g in range(b_shard // gb):
                        cols = slice(g * gb * L, (g + 1) * gb * L)
                        htile = hpool.tile([128, gb * L], F16, tag="h", name="htile")
                        nc.gpsimd.dma_start(out=htile, in_=h_t16[dc, :, cols])
                        hview = htile.rearrange("p (b l) -> p b l", l=L)
                        nc.vector.reduce_sum(
                            sT[:, dc, g * gb:(g + 1) * gb], hview, axis=AX.X
                        )
                sT16 = consts.tile([128, n_dc, b_shard], F16, tag="sT16", name="sT16")
                with nc.allow_low_precision(reason="s matvec runs in fp16 on PE"):
                    nc.vector.tensor_copy(out=sT16, in_=sT)

                # whole W_s^T resident (one 4.2MB DMA, behind h on the gpsimd
                # queue) so the matvec below runs at PE speed, not DMA-paced
                ws_sb = wspool.tile([128, n_dc, D], F16, tag="ws", name="ws_sb")
                nc.gpsimd.dma_start(out=ws_sb, in_=ws_t)

                bias_ps = accpool.tile([b_shard, D], F32, tag="acc", name="bias_ps")
                for ec in range(n_ec):
                    sl = slice(ec * 512, (ec + 1) * 512)
                    for dc in range(n_dc):
                        nc.tensor.matmul(
                            bias_ps[:, sl],
                            sT16[:, dc, :],
                            ws_sb[:, dc, sl],
                            start=(dc == 0),
                            stop=False,
                        )
                    nc.tensor.matmul(
                        bias_ps[:, sl], ones1, bconst_sb[:, sl], start=False, stop=True
                    )

                # bf16 hi/lo split (bias_hi + bias_lo == bias to ~2e-4)
                nc.vector.tensor_copy(out=bias_hi, in_=bias_ps)
                nc.vector.tensor_tensor(
                    out=bias_lo, in0=bias_ps, in1=bias_hi, op=ALU.subtract
                )

                # batch 0 broadcast bias via rank-2 ones-matmul -> f32 SBUF
                b2c = make_bias2_cur(0)
                for half in range(2):
                    bc_ps = zpool.tile([128, 1024], F32, tag="z", name="bc_ps")
                    for ec2 in range(2):
                        sl = slice(ec2 * 512, (ec2 + 1) * 512)
                        gsl = slice(half * 1024 + ec2 * 512, half * 1024 + (ec2 + 1) * 512)
                        nc.tensor.matmul(
                            bc_ps[:, sl], ones2, b2c[:, gsl], start=True, stop=True
                        )
                    nc.scalar.copy(
                        out=bc0[:, half * 1024:(half + 1) * 1024], in_=bc_ps
                    )
                # prefetch the next batches' bias tiles so rb4/rb8 never wait
                for nb in (1, 2):
                    if nb < b_shard:
                        b2cur[nb] = make_bias2_cur(nb)

            def make_bias2_cur(b):
                """Per-batch [2, D] (hi; lo) tile for the rank-2 PSUM fold."""
                b2c = b2pool.tile([2, D], BF16, tag="b2c", name=f"b2c_{b}")
                nc.gpsimd.dma_start(out=b2c[0:1], in_=bias_hi[b:b + 1])
                nc.gpsimd.dma_start(out=b2c[1:2], in_=bias_lo[b:b + 1])
                return b2c

            # ---- helpers ------------------------------------------------
            acc_box = {}
            acc_emitted = [0]

            def emit_acc(rb_, ww_, p_):
                if "acc" not in acc_box:
                    acc_box["acc"] = accpool.tile(
                        [b_shard, D], F32, tag="acc", name="acc_ps"
                    )
                acc_ps = acc_box["acc"]
                acc_emitted[0] += 1
                first = acc_emitted[0] == 1
                last = acc_emitted[0] == n_rb
                for ec in range(n_ec):
                    sl = slice(ec * 512, (ec + 1) * 512)
                    nc.tensor.matmul(
                        acc_ps[:, sl], ww_, p_[:, sl], start=first, stop=last
                    )

            def emit_z_a_mms(z, a_sb, eh, close):
                for kc in range(n_kc):
                    for ec2 in range(2):
                        ec = eh * 2 + ec2
                        nc.tensor.matmul(
                            z[:, ec2 * 512:(ec2 + 1) * 512],
                            a_sb[:, kc, :],
                            w_slice(kc, slice(ec * 512, (ec + 1) * 512)),
                            start=(kc == 0),
                            stop=(close and kc == n_kc - 1),
                        )

            def emit_softmax_tail(b, rb, p_tile, s2):
                s_sum = spool.tile([128, 1], F32, tag="s_sum")
                nc.vector.tensor_tensor(
                    out=s_sum, in0=s2[:, 0:1], in1=s2[:, 1:2], op=ALU.add
                )
                w_rcp = spool.tile([128, 1], F32, tag="w_rcp")
                nc.vector.reciprocal(out=w_rcp, in_=s_sum)
                ww = spool.tile([128, b_shard], BF16, tag="ww")
                nc.vector.memset(ww, 0.0)
                nc.vector.tensor_copy(out=ww[:, b:b + 1], in_=w_rcp)
                return ww

            def exp_half(p_tile, t_in, rb, eh, s2):
                nc.scalar.activation(
                    p_tile[:, eh * 1024:(eh + 1) * 1024],
                    t_in,
                    AF.Exp,
                    scale=mask_sb[:, rb:rb + 1],
                    accum_out=s2[:, eh:eh + 1],
                )

            stash = []
            deferred = []

            def finish_one_stash():
                """Produce p for ONE stashed row block (DVE add + ACT chain).
                Interleaved one-per-rb so the ACT queue never backs up behind
                the stash work (in-order engine streams); the acc matmuls
                drain through the deferred queue."""
                rb_, b_, z_sb = stash.pop(0)
                p_tile = pstash.tile([128, D], BF16, tag="pst", name=f"pst{rb_}")
                s2 = spool.tile([128, 2], F32, tag="s2", name=f"s2st{rb_}")
                for eh in range(2):
                    hs = slice(eh * 1024, (eh + 1) * 1024)
                    ta = tpool.tile([128, 1024], BF16, tag="t", name=f"ta{rb_}{eh}")
                    nc.vector.tensor_tensor(
                        out=ta, in0=z_sb[:, hs], in1=bc0[:, hs], op=ALU.add
                    )
                    t_tile = tpool.tile(
                        [128, 1024], BF16, tag="t", name=f"t{rb_}{eh}"
                    )
                    nc.scalar.activation(t_tile, ta, AF.Tanh)
                    exp_half(p_tile, t_tile, rb_, eh, s2)
                ww = emit_softmax_tail(b_, rb_, p_tile, s2)
                deferred.append((rb_, ww, p_tile))

            # ---- main loop ----------------------------------------------
            b2cur = {}
            for b in range(b_shard):
                for rbl in range(rb_per_batch):
                    rb = b * rb_per_batch + rbl
                    if stash and rb >= n_stash:
                        with nc.named_scope("stashfin"):
                            finish_one_stash()
                    if rbl == 0 and b >= 1 and b not in b2cur:
                        b2cur[b] = make_bias2_cur(b)

                    if rb in a_sb_pre:
                        a_sb = a_sb_pre.pop(rb)
                    else:
                        a_sb = apool.tile([128, n_kc, 128], BF16, tag="a_sb")
                        nc.sync.dma_start(out=a_sb, in_=a_t[rb])

                    if rb < n_stash:
                        # late-bias path: close groups without bias, stash z
                        z_sb = zstash.tile([128, D], F16, tag="zst", name=f"zst{rb}")
                        for eh in range(2):
                            z = zpool.tile([128, 1024], F32, tag="z")
                            emit_z_a_mms(z, a_sb, eh, close=True)
                            nc.scalar.copy(
                                out=z_sb[:, eh * 1024:(eh + 1) * 1024], in_=z
                            )
                        stash.append((rb, b, z_sb))
                        if rb == n_stash - 2:
                            with nc.named_scope("biascomp"):
                                emit_bias_computation()
                        continue

                    p_tile = ppool.tile([128, D], BF16, tag="p")
                    s2 = spool.tile([128, 2], F32, tag="s2")
                    for eh in range(2):
                        z = zpool.tile([128, 1024], F32, tag="z")
                        emit_z_a_mms(z, a_sb, eh, close=False)
                        for ec2 in range(2):
                            ec = eh * 2 + ec2
                            nc.tensor.matmul(
                                z[:, ec2 * 512:(ec2 + 1) * 512],
                                ones2,
                                b2cur[b][:, ec * 512:(ec + 1) * 512],
                                start=False,
                                stop=True,
                            )
                        t_tile = tpool.tile([128, 1024], BF16, tag="t")
                        nc.scalar.activation(t_tile, z, AF.Tanh)
                        exp_half(p_tile, t_tile, rb, eh, s2)

                    ww = emit_softmax_tail(b, rb, p_tile, s2)
                    deferred.append((rb, ww, p_tile))
                    while len(deferred) > 2:
                        emit_acc(*deferred.pop(0))

            while stash:
                finish_one_stash()
            for item in deferred:
                emit_acc(*item)

            # ---- finalize: out = acc * trig_a ---------------------------
            trig_sb = small32.tile([b_shard, D], F32, tag="small32f")
            nc.sync.dma_start(out=trig_sb, in_=trig)
            nc.vector.tensor_tensor(
                out=trig_sb, in0=acc_box["acc"], in1=trig_sb, op=ALU.mult
            )
            nc.sync.dma_start(out=out, in_=trig_sb)

    nc.compile()
    return nc


def prep_core_inputs(h_c, x_c, trigger_c, mask_c, w_t, ws_t, bconst):
    """Host-side data prep for one core's shard (b_shard batches)."""
    b_shard = h_c.shape[0]
    rows = b_shard * L
    n_rb = rows // 128
    n_dc = H // 128

    a_core = np.concatenate(
        [h_c.reshape(rows, H), x_c.reshape(rows, X)], axis=1
    )  # [rows, D] f32

    a_t = np.ascontiguousarray(
        a_core.reshape(n_rb, 128, D // 128, 128).transpose(0, 3, 2, 1)
    ).astype(ml_dtypes.bfloat16)

    h_t16 = np.ascontiguousarray(h_c.reshape(rows, H).T).reshape(n_dc, 128, rows)
    h_t16 = np.ascontiguousarray(h_t16, dtype=np.float16)

    maskf = np.ascontiguousarray(
        mask_c.reshape(rows).reshape(n_rb, 128).T
    ).astype(np.float32)

    tr = np.asarray(trigger_c).reshape(b_shard).astype(np.int64)
    trig = a_core[np.arange(b_shard) * L + tr].astype(np.float32)

    return {
        "a_t": a_t,
        "w_t": w_t,
        "h_t16": h_t16,
        "ws_t": ws_t,
        "bconst": bconst,
        "maskf": maskf,
        "trig": np.ascontiguousarray(trig),
    }


_PROGRAM_CACHE = {}


def build_in_maps(h_state, x, W_a, b_a, W_s, b_s, trigger, mask):
    h_state = np.asarray(h_state, dtype=np.float32)
    x = np.asarray(x, dtype=np.float32)
    W_a = np.asarray(W_a, dtype=np.float32)
    b_a = np.asarray(b_a, dtype=np.float32)
    W_s = np.asarray(W_s, dtype=np.float32)
    b_s = np.asarray(b_s, dtype=np.float32)
    trigger = np.asarray(trigger)
    mask = np.asarray(mask)

    # replicated weights
    w_t = np.ascontiguousarray(
        W_a.T.astype(ml_dtypes.bfloat16).reshape(D // 128, 128, D).transpose(1, 0, 2)
    )
    ws_t = np.ascontiguousarray(
        W_s.T.astype(np.float16).reshape(H // 128, 128, D).transpose(1, 0, 2)
    )
    bconst = np.ascontiguousarray((b_a + b_s).reshape(1, D), dtype=np.float16)

    in_maps = []
    for c in range(N_CORES):
        bs = slice(c * B_SHARD, (c + 1) * B_SHARD)
        in_maps.append(
            prep_core_inputs(
                h_state[bs], x[bs], trigger[bs], mask[bs], w_t, ws_t, bconst
            )
        )
    return in_maps


def kernel(h_state, x, W_a, b_a, W_s, b_s, trigger, mask):
    global LAST_RESULT
    in_maps = build_in_maps(h_state, x, W_a, b_a, W_s, b_s, trigger, mask)

    if B_SHARD not in _PROGRAM_CACHE:
        _PROGRAM_CACHE[B_SHARD] = build_program(B_SHARD)
    nc = _PROGRAM_CACHE[B_SHARD]

    import os
    trace = bool(int(os.environ.get("KERNEL_TRACE", "0")))
    res = run_bass_kernel_spmd(nc, in_maps, core_ids=list(range(N_CORES)), trace=trace)
    LAST_RESULT = res

    out = np.empty((B, D), dtype=np.float32)
    for c in range(N_CORES):
        out[c * B_SHARD:(c + 1) * B_SHARD] = res.results[c]["out"]
    return out
